# revision 4
# baseline (speedup 1.0000x reference)
"""Trainium2 Bass kernel v7 for the 2D Gaussian splatting model (nn_GaussianModel2D).

Math (per pixel p, gaussians n = 0..255 in order):
    e_n(p)   = -(a dx^2 + 2b dx dy + c dy^2) + ln(opac_n)      (quadratic in x,y)
    alpha_n  = exp(e_n)        (clip at 0.99 never binds for this input;
                                host-checked, fallback program applies it)
    u_n      = 1 - alpha_n
    T_t      = prod_{k<=t} u_k (inclusive scan)
    out_c    = c0_c + sum_t gamma_{t,c} * T_t      (host clips to [0,1])
    gamma_t = col_{t+1}-col_t (t<255), gamma_255 = 1-col_255   (Abel summation)

v2 layout changes vs v1 (all per core = 1/8 of rows = 32768 pixels):
    - supergroups of 8 chunks x 128 pixels; alpha/u/T live in a gapped fp16
      stream [128, 8, 272] = 256 data + 16 tail slots per block.
    - ONE tensor_tensor_scan per supergroup (FD=2176) instead of 8 FD=256
      calls: the tail slots carry (d0=0, d1=1) which resets state to 1.0
      between the 8 pixels chained on each partition lane, amortizing the
      DVE per-instruction overhead.
    - alpha in fp16 (ACT exp writes fp16), u-pass tensor_scalar in fp16
      (eligible for the DVE 4x mode), scan fp16 (eligible for 2x).
    - dma_start_transpose with 3-D out [128, 2, 128]: both 128-col halves
      of a block in one instruction.
    - render matmuls: gamma loaded 2x per supergroup (not per chunk), 4
      col-groups run concurrently via tile_position=(0, 32m).
    - +c0 folded into the PSUM->SBUF merge (tensor_scalar bias); final
      clip to [0,1] on host.
"""

import numpy as np

H, W, N = 512, 512, 256
NCORES = 8
ROWS_PER_CORE = H // NCORES            # 64
PIX = ROWS_PER_CORE * W                # 32768 pixels per core
CHUNK = 128                            # pixels per matmul chunk
SG = 8                                 # chunks per supergroup
SGPIX = SG * CHUNK                     # 1024
NSG = PIX // SGPIX                     # 32
BLK = 272                              # 256 data + 16 reset-tail slots
FD = SG * BLK                          # 2176 scan stream length
OSG = 4                                # supergroups per output staging tile

_CACHE = {}


def _build_program(apply_opacity_clip: bool):
    import concourse.bass as bass
    import concourse.bacc as bacc
    import concourse.tile as tile
    import concourse.mybir as mybir
    from contextlib import ExitStack

    fp32 = mybir.dt.float32
    fp32r = mybir.dt.float32r
    fp16 = mybir.dt.float16
    Alu = mybir.AluOpType
    Act = mybir.ActivationFunctionType

    nc = bacc.Bacc("TRN2", target_bir_lowering=False, debug=False,
                   num_devices=NCORES)

    ft_d = nc.dram_tensor("ft", [6, PIX], fp32r, kind="ExternalInput")
    c6_d = nc.dram_tensor("c6", [6, N], fp32r, kind="ExternalInput")
    gsh_d = nc.dram_tensor("gsh", [128, SG * 3 * 32], fp16, kind="ExternalInput")
    c0_d = nc.dram_tensor("c0", [128, 1], fp32, kind="ExternalInput")
    out_d = nc.dram_tensor("out", [3, PIX], fp32, kind="ExternalOutput")

    with tile.TileContext(nc) as tc, ExitStack() as ctx:
        consts = ctx.enter_context(tc.tile_pool(name="consts", bufs=1))
        alp = ctx.enter_context(tc.tile_pool(name="alp", bufs=2))
        t2p = ctx.enter_context(tc.tile_pool(name="t2p", bufs=2))
        ttp = ctx.enter_context(tc.tile_pool(name="ttp", bufs=2))
        osbp = ctx.enter_context(tc.tile_pool(name="osbp", bufs=2))
        epsp = ctx.enter_context(tc.tile_pool(name="epsp", bufs=3, space="PSUM"))
        rpsp = ctx.enter_context(tc.tile_pool(name="rpsp", bufs=2, space="PSUM"))

        ft_sb = consts.tile([6, PIX], fp32r)
        c6_sb = consts.tile([6, N], fp32r)
        gsh_sb = consts.tile([128, SG, 3, 32], fp16)
        c0_sb = consts.tile([128, 1], fp32)
        d1_sb = consts.tile([128, SG, BLK], fp16)
        # persistent double-buffered u stream; gap slots must stay 0
        u2_a = consts.tile([128, SG, BLK], fp16)
        u2_b = consts.tile([128, SG, BLK], fp16)

        nc.sync.dma_start(ft_sb[:], ft_d[:])
        nc.sync.dma_start(c6_sb[:], c6_d[:])
        nc.sync.dma_start(gsh_sb[:], gsh_d.ap().rearrange(
            "p (a k c) -> p a k c", a=SG, k=3, c=32))
        nc.sync.dma_start(c0_sb[:], c0_d[:])
        nc.vector.memset(d1_sb[:], 0.0)
        nc.vector.memset(d1_sb[:, :, 256:BLK], 1.0)
        nc.vector.memset(u2_a[:], 0.0)
        nc.vector.memset(u2_b[:], 0.0)

        u2_bufs = [u2_a, u2_b]

        for i in range(NSG):
            u2 = u2_bufs[i % 2]
            al_t = alp.tile([128, SG, BLK], fp16)
            for h in range(2):
                e_ps = epsp.tile([128, 1024], fp32)
                for q in range(4):
                    j = SG * i + 4 * h + q
                    nc.tensor.matmul(
                        e_ps[:, 256 * q:256 * (q + 1)],
                        lhsT=ft_sb[:, CHUNK * j:CHUNK * (j + 1)],
                        rhs=c6_sb[:],
                        start=True, stop=True)
                nc.scalar.activation(al_t[:, 4 * h:4 * h + 4, 0:256], e_ps[:],
                                     Act.Exp)
            # u = 1 - alpha on the data slots only (gaps stay 0)
            nc.vector.tensor_scalar(u2[:, :, 0:256], al_t[:, :, 0:256],
                                    -1.0, 1.0, Alu.mult, Alu.add)
            if apply_opacity_clip:
                # u = max(1 - alpha, 0.01)  ==  1 - min(alpha, 0.99)
                nc.vector.tensor_scalar(u2[:, :, 0:256], u2[:, :, 0:256],
                                        0.01, None, Alu.max)
            # inclusive cumprod with per-block reset via (d0=0, d1=1) tails
            t2_t = t2p.tile([128, SG, BLK], fp16)
            nc.vector.tensor_tensor_scan(
                t2_t.rearrange("p a b -> p (a b)"),
                data0=u2.rearrange("p a b -> p (a b)"),
                data1=d1_sb.rearrange("p a b -> p (a b)"),
                initial=1.0, op0=Alu.mult, op1=Alu.add)
            # ONE batched transpose of the whole gapped stream (17 slabs)
            tt_t = ttp.tile([128, FD // CHUNK, CHUNK], fp16)
            nc.sync.dma_start_transpose(tt_t[:],
                                        t2_t.rearrange("p a b -> p (a b)"))
            # render: zero-padded shifted-gamma pieces; block a starts at
            # slab 2a with partition shift sigma = 16a
            r_ps = rpsp.tile([128, 256], fp32)
            for a in range(SG):
                m, s = a % 4, a // 4
                sigma = (BLK * a) % 128
                nk = 2 if sigma == 0 else 3
                for k in range(nk):
                    nc.tensor.matmul(
                        r_ps[32 * m:32 * m + 32, CHUNK * s:CHUNK * (s + 1)],
                        lhsT=gsh_sb[:, a, k, :],
                        rhs=tt_t[:, 2 * a + k, :],
                        start=(k == 0), stop=(k == nk - 1),
                        tile_position=(0, 32 * m))
            # merge (+c0 bias) into the output staging tile
            if i % OSG == 0:
                o_sb = osbp.tile([128, OSG * 256], fp32)
            sl = (i % OSG) * 256
            nc.vector.tensor_scalar(o_sb[:, sl:sl + 256], r_ps[:],
                                    c0_sb[:], None, Alu.add)
            if i % OSG == OSG - 1:
                # out[c, 4096*g + 1024*di + 128*(m + 4*s) + x]
                #   = o_sb[32m+c, 256*di + 128*s + x]
                dst6 = out_d.ap().rearrange(
                    "c (g di s m x) -> c g di s m x",
                    g=NSG // OSG, di=OSG, s=2, m=4, x=CHUNK)
                for m in range(4):
                    nc.sync.dma_start(
                        dst6[:, i // OSG, :, :, m, :],
                        o_sb[32 * m:32 * m + 3, :].rearrange(
                            "c (di s x) -> c di s x", di=OSG, s=2, x=CHUNK))
    nc.compile()
    return nc


def _prep_host(coords, means, log_scales, rotations, raw_colors, raw_opacities):
    """Tiny host-side parameter preparation (float64 for coefficient accuracy)."""
    f64 = np.float64
    scales = np.exp(log_scales.astype(f64))
    sx2, sy2 = scales[:, 0] ** 2, scales[:, 1] ** 2
    cos_r = np.cos(rotations.astype(f64))
    sin_r = np.sin(rotations.astype(f64))
    a = cos_r ** 2 / (2 * sx2) + sin_r ** 2 / (2 * sy2)
    b = -sin_r * cos_r / (2 * sx2) + sin_r * cos_r / (2 * sy2)
    c = sin_r ** 2 / (2 * sx2) + cos_r ** 2 / (2 * sy2)
    opac = 1.0 / (1.0 + np.exp(-raw_opacities.astype(f64)))
    colors = 1.0 / (1.0 + np.exp(-raw_colors.astype(f64)))   # (N, 3)

    mx = means[:, 0].astype(f64) - 0.5
    my = means[:, 1].astype(f64) - 0.5
    # e = -(a dx^2 + 2 b dx dy + c dy^2) + ln(opac), expanded over centered
    # features [x^2, xy, y^2, x, y, 1]
    C6 = np.stack([
        -a,
        -2.0 * b,
        -c,
        2.0 * a * mx + 2.0 * b * my,
        2.0 * b * mx + 2.0 * c * my,
        -(a * mx ** 2 + 2.0 * b * mx * my + c * my ** 2) + np.log(opac),
    ]).astype(np.float32)                                    # (6, N)

    gam = np.zeros((N, 3), f64)
    gam[:N - 1] = colors[1:] - colors[:-1]
    gam[N - 1] = 1.0 - colors[N - 1]
    gsh = np.zeros((128, SG, 3, 32), f64)
    pi = np.arange(128)
    for blk in range(SG):
        sigma = (BLK * blk) % 128
        w = np.zeros((128, 3)); sel = pi >= sigma
        w[sel] = gam[pi[sel] - sigma]
        gsh[:, blk, 0, :3] = w
        gsh[:, blk, 1, :3] = gam[128 - sigma + pi]
        w = np.zeros((128, 3)); sel = pi < sigma
        w[sel] = gam[256 - sigma + pi[sel]]
        gsh[:, blk, 2, :3] = w
    gam = gsh.reshape(128, SG * 3 * 32).astype(np.float16)

    c0 = np.zeros((128, 1), np.float32)
    for m in range(4):
        c0[32 * m:32 * m + 3, 0] = colors[0].astype(np.float32)

    x = coords[:, :, 0].astype(f64) - 0.5                    # (H, W)
    y = coords[:, :, 1].astype(f64) - 0.5
    feats = np.stack([x * x, x * y, y * y, x, y, np.ones_like(x)])  # (6, H, W)
    feats = feats.reshape(6, NCORES, PIX).astype(np.float32)

    clip_needed = bool((opac > 0.99).any())
    return feats, C6, gam, c0, clip_needed


def _program_and_inmaps(prep):
    feats, C6, gam, c0, clip_needed = prep
    key = ("prog", clip_needed)
    if key not in _CACHE:
        _CACHE[key] = _build_program(clip_needed)
    nc = _CACHE[key]
    in_maps = [
        {"ft": np.ascontiguousarray(feats[:, k]), "c6": C6, "gsh": gam, "c0": c0}
        for k in range(NCORES)
    ]
    return nc, in_maps


def kernel(coords, means, log_scales, rotations, raw_colors, raw_opacities):
    from concourse.bass_utils import run_bass_kernel_spmd

    prep = _prep_host(
        coords, means, log_scales, rotations, raw_colors, raw_opacities)
    nc, in_maps = _program_and_inmaps(prep)
    res = run_bass_kernel_spmd(nc, in_maps, list(range(NCORES)))
    out = np.stack([res.results[k]["out"] for k in range(NCORES)])  # (8, 3, PIX)
    out = out.reshape(NCORES, 3, ROWS_PER_CORE, W).transpose(0, 2, 3, 1)
    out = out.reshape(H, W, 3)
    return np.clip(out, 0.0, 1.0).astype(np.float32)
